# revision 35
# baseline (speedup 1.0000x reference)
"""MoE-LoRA kernel for Trainium2 (8 NeuronCores, Bass/Tile).

Math: per sample b (except the last), with label e = label[b]:
    out[b] = ALPHA * ( (x[b] @ A_e.T) @ B_e.T  +  (x[b] @ A_gen.T) @ B_gen.T )
The expert and general LoRA paths merge into a single rank-128 LoRA:
    Acat[b] = [A_e ; A_gen]           [2R, D]
    Bcat[b] = [B_e , B_gen]           [D, 2R]
    out[b]  = (x[b] @ Acat[b].T) @ (ALPHA * Bcat[b]).T

Work unit: one 512-row S-block of one sample. Sample B-1 is dead (the torch
loop runs range(B-1)), so there are 31*8 = 248 live blocks = 31 per core.
Every core runs the same program: 31 blocks against 6 table slots with the
static pattern [8,8,8,4,2,1]; the host maps (core, j) -> (sample, S-block).

x is pre-transposed on the host (bf16), so the device pipeline is just:
    DMA xT half-blocks (partition-major, 5 KB/partition contiguous)
    PE  GEMM1: hT[2R, S] = AcatT.T @ xT        (accumulated over 10 D chunks)
    DVE evacuate hT -> SBUF float32r
    PE  GEMM2: out[S, D] = hT.T @ bcatT        (float32r, single K=128)
    ACT/DVE evacuate out tiles -> SBUF bf16, ScalarE-issued DMA out
No PE transposes (the old baseline spent ~1/3 of PE time there). The kernel
is HBM-bound: ~81 MB/core at ~358 GB/s. The in-DMA is split into 2 halves
per block so PE idle gaps stay under the ~3.4us HAM re-throttle window.
"""

import numpy as np
import ml_dtypes

import concourse.mybir as mybir
import concourse.tile as tile
from concourse import bacc
from concourse.bass import ts
from concourse.bass_utils import run_bass_kernel_spmd

# Problem shape (hardcoded; kernel.py must be self-contained).
B, S, D, R, E = 32, 4096, 1280, 64, 8
ALPHA = 2.0
NCORES = 8
R2 = 2 * R                # merged LoRA rank = 128
P = 128
SBK = 512                 # S rows per block
NSB = S // SBK            # 8 S-blocks per sample
NST = SBK // P            # 4 S-subtiles per block
DC = D // P               # 10 D chunks
DCH = DC // 2             # D chunks per DMA half
NBLK = 31                 # blocks per core
NSLOT = 6                 # table slots per core
# block j uses table slot SLOT_OF[j]; same for every core
SLOT_OF = [0] * 8 + [1] * 8 + [2] * 8 + [3] * 4 + [4] * 2 + [5]

F32 = mybir.dt.float32
F32R = mybir.dt.float32r
BF16 = mybir.dt.bfloat16

_CACHED = {}


def _blocks_for_core(c):
    """(sample, sbi) list for core c, matching SLOT_OF's [8,8,8,4,2,1]."""
    bl = []
    for t in range(3):
        bl += [(3 * c + t, s) for s in range(NSB)]
    s0 = 4 * (c % 2)
    bl += [(24 + c // 2, s) for s in range(s0, s0 + 4)]
    s0 = 2 * (c % 4)
    bl += [(28 + c // 4, s) for s in range(s0, s0 + 2)]
    bl.append((30, c))
    return bl


def _slot_samples(c):
    return [3 * c, 3 * c + 1, 3 * c + 2, 24 + c // 2, 28 + c // 4, 30]


def _build_module():
    nc = bacc.Bacc(None, target_bir_lowering=False)

    # xt[j] = [128 d-part, 10 d-chunk, 512 s]  (partition-major, contiguous)
    xt_d = nc.dram_tensor("xt", [NBLK, P, DC, SBK], BF16, kind="ExternalInput")
    # acat[p, t, k, r]: table slot t, D chunk k -> AcatT tile [128 d-part, 128 r]
    acat_d = nc.dram_tensor("acat", [P, NSLOT, DC, R2], BF16, kind="ExternalInput")
    # bcat[p, t, d]: (ALPHA * Bcat).T per slot   [128 r, 1280 d]
    bcat_d = nc.dram_tensor("bcat", [P, NSLOT, D], BF16, kind="ExternalInput")
    # paired out layout: 15 pairs of blocks + the final single block, all
    # partition-major so each out-DMA is one 20KB (10KB) contiguous run per
    # partition with no AP rearrange
    outp_d = nc.dram_tensor(
        "outp", [NBLK // 2, P, 2, NST, D], BF16, kind="ExternalOutput"
    )
    outl_d = nc.dram_tensor("outl", [P, NST, D], BF16, kind="ExternalOutput")

    with tile.TileContext(nc) as tc:
        with (
            tc.tile_pool(name="const", bufs=1) as constp,
            tc.tile_pool(name="xt", bufs=6) as xt_p,
            tc.tile_pool(name="ht", bufs=3) as ht_p,
            tc.tile_pool(name="osb", bufs=3) as out_p,
            tc.tile_pool(name="h_ps", bufs=2, space="PSUM") as h_ps,
            tc.tile_pool(name="o_ps", bufs=6, space="PSUM") as o_ps,
        ):
            acat_sb = constp.tile([P, NSLOT, DC, R2], BF16)
            bcat_sb = constp.tile([P, NSLOT, D], BF16)
            # table loads ride the scalar (out) ring, which is otherwise idle
            # until the first out-pair (~20us); per-slot pieces so slot 0
            # lands in ~4us and block 0 compute starts immediately
            for t in range(NSLOT):
                nc.scalar.dma_start(acat_sb[:, t], acat_d[:, t])
                nc.scalar.dma_start(bcat_sb[:, t], bcat_d[:, t])

            # warm the PE's HAM clock gate (~3.4us of matmuls) while the
            # first xT block is still in flight, so block 0 runs at 2.4GHz
            wsrc = constp.tile([P, SBK], BF16)
            nc.vector.memset(wsrc[:], 0.0)
            wp = h_ps.tile([P, SBK], F32, tag="hp")
            for _ in range(16):
                nc.tensor.matmul(wp[:], wsrc[:, :P], wsrc[:], start=True, stop=True)

            out_sb = None
            for j in range(NBLK):
                sl = SLOT_OF[j]
                # whole-block loads, alternating between the sync HWDGE ring
                # and the gpsimd SWDGE queue: two lanes deliver ~2x the
                # per-ring rate (each dma_start pays ~1.8-2.5us serialized
                # ring overhead), so the in-stream outruns PE consumption
                xt = xt_p.tile([P, DC, SBK], BF16, tag="xt")
                eng = nc.sync if j % 2 == 0 else nc.gpsimd
                if j < 8:
                    # ramp: halves let GEMM1 start on the first 5 D-chunks
                    # while the rest of the block is still in flight
                    eng.dma_start(xt[:, :DCH], xt_d[j, :, :DCH])
                    eng.dma_start(xt[:, DCH:], xt_d[j, :, DCH:])
                else:
                    eng.dma_start(xt[:], xt_d[j])

                # GEMM1: hT[r, s] accumulated over D chunks
                hp = h_ps.tile([P, SBK], F32, tag="hp")
                for k in range(DC):
                    nc.tensor.matmul(
                        hp[:],
                        acat_sb[:, sl, k],
                        xt[:, k],
                        start=(k == 0),
                        stop=(k == DC - 1),
                    )
                ht = ht_p.tile([P, SBK], BF16, tag="ht")
                nc.vector.tensor_copy(ht[:], hp[:])

                # GEMM2: out[s, d] = hT.T @ bcatT; ACT/DVE evacuate evenly.
                # out_sb covers TWO blocks: per-dma_start overhead caps the
                # out ring near ~170GB/s at 1.31MB; 2.62MB pairs ~275GB/s.
                half = j % 2
                if half == 0:
                    out_sb = out_p.tile([P, 2, NST, D], BF16, tag="out_sb")
                for st in range(NST):
                    for nb in range(3):
                        n0 = nb * 512
                        nsz = 512 if nb < 2 else 256
                        op = o_ps.tile([P, 512], F32, tag="op")
                        nc.tensor.matmul(
                            op[:, :nsz],
                            ht[:, ts(st, P)],
                            bcat_sb[:, sl, n0 : n0 + nsz],
                            start=True,
                            stop=True,
                        )
                        on_act = nb == 0 or (nb == 2 and st < 2)
                        if on_act:
                            nc.scalar.copy(
                                out_sb[:, half, st, n0 : n0 + nsz], op[:, :nsz]
                            )
                        else:
                            nc.vector.tensor_copy(
                                out_sb[:, half, st, n0 : n0 + nsz], op[:, :nsz]
                            )

                # out-DMA from ScalarE (HWDGE ring separate from sync's);
                # the last pairs ride the sync ring, which is idle once the
                # in-stream finishes, so the tail drains on two rings
                if half == 1:
                    oeng = nc.sync if (j // 2) % 2 == 1 else nc.scalar
                    oeng.dma_start(outp_d[j // 2], out_sb[:])
                elif j == NBLK - 1:
                    nc.scalar.dma_start(outl_d[:], out_sb[:, 0])

    nc.finalize()
    return nc


def _get_module():
    if "m" not in _CACHED:
        _CACHED["m"] = _build_module()
    return _CACHED["m"]


def _prepare_in_maps(x, weight, A_experts, B_experts, A_gen, B_gen, label):
    x = np.ascontiguousarray(np.asarray(x), dtype=np.float32)
    A_experts = np.asarray(A_experts, dtype=np.float32)
    B_experts = np.asarray(B_experts, dtype=np.float32)
    A_gen = np.asarray(A_gen, dtype=np.float32)
    B_gen = np.asarray(B_gen, dtype=np.float32)
    label = np.asarray(label).astype(np.int64)

    Ae = A_experts[label]                                   # [B, R, D]
    Be = B_experts[label]                                   # [B, D, R]
    Acat = np.concatenate(
        [Ae, np.broadcast_to(A_gen, (B, R, D))], axis=1
    )                                                       # [B, 2R, D]
    Bcat = np.concatenate(
        [Be, np.broadcast_to(B_gen, (B, D, R))], axis=2
    )                                                       # [B, D, 2R]
    acatT = Acat.transpose(0, 2, 1).reshape(B, DC, P, R2)   # [B, k, p, r]
    acatT = acatT.astype(ml_dtypes.bfloat16)
    bcatT = np.ascontiguousarray(
        (ALPHA * Bcat).transpose(0, 2, 1)
    ).astype(ml_dtypes.bfloat16)                            # [B, 2R, D]

    xb = x.astype(ml_dtypes.bfloat16)
    xb5 = xb.reshape(B, NSB, SBK, DC, P)                    # [b, sbi, s, k, p]

    in_maps = []
    for c in range(NCORES):
        bs = _blocks_for_core(c)
        barr = np.array([b for b, _ in bs])
        sarr = np.array([s for _, s in bs])
        blk = xb5[barr, sarr]                               # [31, 512, 10, 128]
        xt = np.ascontiguousarray(
            blk.transpose(0, 3, 2, 1)
        )                                                   # [31, 128, 10, 512]
        sl = _slot_samples(c)
        acat_c = np.ascontiguousarray(acatT[sl].transpose(2, 0, 1, 3))
        bcat_c = np.ascontiguousarray(bcatT[sl].transpose(1, 0, 2))
        in_maps.append({"xt": xt, "acat": acat_c, "bcat": bcat_c})
    return in_maps


def _run(trace=False, **inputs):
    nc = _get_module()
    in_maps = _prepare_in_maps(**inputs)
    res = run_bass_kernel_spmd(
        nc, in_maps, core_ids=list(range(NCORES)), trace=trace
    )
    out = np.zeros((B, S, D), dtype=np.float32)
    o4 = out.reshape(B, NSB, SBK, D)
    for c in range(NCORES):
        op = np.asarray(res.results[c]["outp"]).astype(np.float32)
        ol = np.asarray(res.results[c]["outl"]).astype(np.float32)
        oc = np.concatenate(
            [op.transpose(0, 2, 1, 3, 4).reshape(NBLK - 1, P, NST, D), ol[None]],
            axis=0,
        )
        # [31, p, st, d] -> rows s = st*128 + p
        oc = oc.transpose(0, 2, 1, 3).reshape(NBLK, SBK, D)
        bs = _blocks_for_core(c)
        barr = np.array([b for b, _ in bs])
        sarr = np.array([s for _, s in bs])
        o4[barr, sarr] = oc
    return out, res


def kernel(**inputs) -> np.ndarray:
    out, _ = _run(trace=False, **inputs)
    return out


def kernel_traced(mode=None, **inputs):
    """Returns (out, BassKernelResults) with HW profile info."""
    return _run(trace=True, **inputs)


# revision 37
# speedup vs baseline: 1.1051x; 1.1051x over previous
"""MoE-LoRA kernel for Trainium2 (8 NeuronCores, Bass/Tile).

Math: per sample b (except the last), with label e = label[b]:
    out[b] = ALPHA * ( (x[b] @ A_e.T) @ B_e.T  +  (x[b] @ A_gen.T) @ B_gen.T )
The expert and general LoRA paths merge into a single rank-128 LoRA:
    Acat[b] = [A_e ; A_gen]           [2R, D]
    Bcat[b] = [B_e , B_gen]           [D, 2R]
    out[b]  = (x[b] @ Acat[b].T) @ (ALPHA * Bcat[b]).T

Work unit: one 512-row S-block of one sample. Sample B-1 is dead (the torch
loop runs range(B-1)), so there are 31*8 = 248 live blocks = 31 per core.
Every core runs the same program: 31 blocks against 6 table slots with the
static pattern [8,8,8,4,2,1]; the host maps (core, j) -> (sample, S-block).

x is pre-transposed on the host (bf16), so the device pipeline is just:
    DMA xT half-blocks (partition-major, 5 KB/partition contiguous)
    PE  GEMM1: hT[2R, S] = AcatT.T @ xT        (accumulated over 10 D chunks)
    DVE evacuate hT -> SBUF float32r
    PE  GEMM2: out[S, D] = hT.T @ bcatT        (float32r, single K=128)
    ACT/DVE evacuate out tiles -> SBUF bf16, ScalarE-issued DMA out
No PE transposes (the old baseline spent ~1/3 of PE time there). The kernel
is HBM-bound: ~81 MB/core at ~358 GB/s. The in-DMA is split into 2 halves
per block so PE idle gaps stay under the ~3.4us HAM re-throttle window.
"""

import numpy as np
import ml_dtypes

import concourse.mybir as mybir
import concourse.tile as tile
from concourse import bacc
from concourse.bass import ts
from concourse.bass_utils import run_bass_kernel_spmd

# Problem shape (hardcoded; kernel.py must be self-contained).
B, S, D, R, E = 32, 4096, 1280, 64, 8
ALPHA = 2.0
NCORES = 8
R2 = 2 * R                # merged LoRA rank = 128
P = 128
SBK = 512                 # S rows per block
NSB = S // SBK            # 8 S-blocks per sample
NST = SBK // P            # 4 S-subtiles per block
DC = D // P               # 10 D chunks
DCH = DC // 2             # D chunks per DMA half
NBLK = 31                 # blocks per core
NSLOT = 6                 # table slots per core
# block j uses table slot SLOT_OF[j]; same for every core
SLOT_OF = [0] * 8 + [1] * 8 + [2] * 8 + [3] * 4 + [4] * 2 + [5]

F32 = mybir.dt.float32
F32R = mybir.dt.float32r
BF16 = mybir.dt.bfloat16

_CACHED = {}


def _blocks_for_core(c):
    """(sample, sbi) list for core c, matching SLOT_OF's [8,8,8,4,2,1]."""
    bl = []
    for t in range(3):
        bl += [(3 * c + t, s) for s in range(NSB)]
    s0 = 4 * (c % 2)
    bl += [(24 + c // 2, s) for s in range(s0, s0 + 4)]
    s0 = 2 * (c % 4)
    bl += [(28 + c // 4, s) for s in range(s0, s0 + 2)]
    bl.append((30, c))
    return bl


def _slot_samples(c):
    return [3 * c, 3 * c + 1, 3 * c + 2, 24 + c // 2, 28 + c // 4, 30]


def _build_module():
    nc = bacc.Bacc(None, target_bir_lowering=False)

    # xt[j] = [128 d-part, 10 d-chunk, 512 s]  (partition-major, contiguous)
    xt_d = nc.dram_tensor("xt", [NBLK, P, DC, SBK], BF16, kind="ExternalInput")
    # acat[p, t, k, r]: table slot t, D chunk k -> AcatT tile [128 d-part, 128 r]
    acat_d = nc.dram_tensor("acat", [P, NSLOT, DC, R2], BF16, kind="ExternalInput")
    # bcat[p, t, d]: (ALPHA * Bcat).T per slot   [128 r, 1280 d]
    bcat_d = nc.dram_tensor("bcat", [P, NSLOT, D], BF16, kind="ExternalInput")
    # paired out layout: 15 pairs of blocks + the final single block, all
    # partition-major so each out-DMA is one 20KB (10KB) contiguous run per
    # partition with no AP rearrange
    outp_d = nc.dram_tensor(
        "outp", [NBLK // 2, P, 2, NST, D], BF16, kind="ExternalOutput"
    )
    outl_d = nc.dram_tensor("outl", [P, NST, D], BF16, kind="ExternalOutput")

    with tile.TileContext(nc) as tc:
        with (
            tc.tile_pool(name="const", bufs=1) as constp,
            tc.tile_pool(name="xt", bufs=6) as xt_p,
            tc.tile_pool(name="ht", bufs=3) as ht_p,
            tc.tile_pool(name="osb", bufs=3) as out_p,
            tc.tile_pool(name="h_ps", bufs=2, space="PSUM") as h_ps,
            tc.tile_pool(name="o_ps", bufs=6, space="PSUM") as o_ps,
        ):
            acat_sb = constp.tile([P, NSLOT, DC, R2], BF16)
            bcat_sb = constp.tile([P, NSLOT, D], BF16)
            # table loads ride the scalar (out) ring, which is otherwise idle
            # until the first out-pair (~20us); per-slot pieces so slot 0
            # lands in ~4us and block 0 compute starts immediately
            for t in range(NSLOT):
                nc.scalar.dma_start(acat_sb[:, t], acat_d[:, t])
                nc.scalar.dma_start(bcat_sb[:, t], bcat_d[:, t])

            # warm the PE's HAM clock gate (~3.4us of matmuls) while the
            # first xT block is still in flight, so block 0 runs at 2.4GHz
            wsrc = constp.tile([P, SBK], BF16)
            nc.vector.memset(wsrc[:], 0.0)
            wp = h_ps.tile([P, SBK], F32, tag="hp")
            for _ in range(16):
                nc.tensor.matmul(wp[:], wsrc[:, :P], wsrc[:], start=True, stop=True)

            out_sb = None
            for j in range(NBLK):
                sl = SLOT_OF[j]
                # whole-block loads, alternating between the sync HWDGE ring
                # and the gpsimd SWDGE queue: two lanes deliver ~2x the
                # per-ring rate (each dma_start pays ~1.8-2.5us serialized
                # ring overhead), so the in-stream outruns PE consumption
                xt = xt_p.tile([P, DC, SBK], BF16, tag="xt")
                eng = nc.sync if j % 2 == 0 else nc.gpsimd
                if j < 4:
                    # ramp: halves let GEMM1 start on the first 5 D-chunks
                    # while the rest of the block is still in flight
                    eng.dma_start(xt[:, :DCH], xt_d[j, :, :DCH])
                    eng.dma_start(xt[:, DCH:], xt_d[j, :, DCH:])
                else:
                    eng.dma_start(xt[:], xt_d[j])

                # GEMM1: hT[r, s] accumulated over D chunks
                hp = h_ps.tile([P, SBK], F32, tag="hp")
                for k in range(DC):
                    nc.tensor.matmul(
                        hp[:],
                        acat_sb[:, sl, k],
                        xt[:, k],
                        start=(k == 0),
                        stop=(k == DC - 1),
                    )
                ht = ht_p.tile([P, SBK], BF16, tag="ht")
                nc.vector.tensor_copy(ht[:], hp[:])

                # GEMM2: out[s, d] = hT.T @ bcatT; ACT/DVE evacuate evenly.
                # out_sb covers TWO blocks: per-dma_start overhead caps the
                # out ring near ~170GB/s at 1.31MB; 2.62MB pairs ~275GB/s.
                half = j % 2
                if half == 0:
                    out_sb = out_p.tile([P, 2, NST, D], BF16, tag="out_sb")
                for st in range(NST):
                    for nb in range(3):
                        n0 = nb * 512
                        nsz = 512 if nb < 2 else 256
                        op = o_ps.tile([P, 512], F32, tag="op")
                        nc.tensor.matmul(
                            op[:, :nsz],
                            ht[:, ts(st, P)],
                            bcat_sb[:, sl, n0 : n0 + nsz],
                            start=True,
                            stop=True,
                        )
                        on_act = nb == 0 or (nb == 2 and st < 2)
                        if on_act:
                            nc.scalar.copy(
                                out_sb[:, half, st, n0 : n0 + nsz], op[:, :nsz]
                            )
                        else:
                            nc.vector.tensor_copy(
                                out_sb[:, half, st, n0 : n0 + nsz], op[:, :nsz]
                            )

                # out-DMA from ScalarE (HWDGE ring separate from sync's);
                # the last pairs ride the sync ring, which is idle once the
                # in-stream finishes, so the tail drains on two rings
                if half == 1:
                    oeng = nc.sync if j // 2 >= 11 else nc.scalar
                    oeng.dma_start(outp_d[j // 2], out_sb[:])
                elif j == NBLK - 1:
                    # tail: the final single block drains on the otherwise
                    # idle SWDGE queue, in parallel with the last pairs
                    nc.gpsimd.dma_start(outl_d[:], out_sb[:, 0])

    nc.finalize()
    return nc


def _get_module():
    if "m" not in _CACHED:
        _CACHED["m"] = _build_module()
    return _CACHED["m"]


def _prepare_in_maps(x, weight, A_experts, B_experts, A_gen, B_gen, label):
    x = np.ascontiguousarray(np.asarray(x), dtype=np.float32)
    A_experts = np.asarray(A_experts, dtype=np.float32)
    B_experts = np.asarray(B_experts, dtype=np.float32)
    A_gen = np.asarray(A_gen, dtype=np.float32)
    B_gen = np.asarray(B_gen, dtype=np.float32)
    label = np.asarray(label).astype(np.int64)

    Ae = A_experts[label]                                   # [B, R, D]
    Be = B_experts[label]                                   # [B, D, R]
    Acat = np.concatenate(
        [Ae, np.broadcast_to(A_gen, (B, R, D))], axis=1
    )                                                       # [B, 2R, D]
    Bcat = np.concatenate(
        [Be, np.broadcast_to(B_gen, (B, D, R))], axis=2
    )                                                       # [B, D, 2R]
    acatT = Acat.transpose(0, 2, 1).reshape(B, DC, P, R2)   # [B, k, p, r]
    acatT = acatT.astype(ml_dtypes.bfloat16)
    bcatT = np.ascontiguousarray(
        (ALPHA * Bcat).transpose(0, 2, 1)
    ).astype(ml_dtypes.bfloat16)                            # [B, 2R, D]

    xb = x.astype(ml_dtypes.bfloat16)
    xb5 = xb.reshape(B, NSB, SBK, DC, P)                    # [b, sbi, s, k, p]

    in_maps = []
    for c in range(NCORES):
        bs = _blocks_for_core(c)
        barr = np.array([b for b, _ in bs])
        sarr = np.array([s for _, s in bs])
        blk = xb5[barr, sarr]                               # [31, 512, 10, 128]
        xt = np.ascontiguousarray(
            blk.transpose(0, 3, 2, 1)
        )                                                   # [31, 128, 10, 512]
        sl = _slot_samples(c)
        acat_c = np.ascontiguousarray(acatT[sl].transpose(2, 0, 1, 3))
        bcat_c = np.ascontiguousarray(bcatT[sl].transpose(1, 0, 2))
        in_maps.append({"xt": xt, "acat": acat_c, "bcat": bcat_c})
    return in_maps


def _run(trace=False, **inputs):
    nc = _get_module()
    in_maps = _prepare_in_maps(**inputs)
    res = run_bass_kernel_spmd(
        nc, in_maps, core_ids=list(range(NCORES)), trace=trace
    )
    out = np.zeros((B, S, D), dtype=np.float32)
    o4 = out.reshape(B, NSB, SBK, D)
    for c in range(NCORES):
        op = np.asarray(res.results[c]["outp"]).astype(np.float32)
        ol = np.asarray(res.results[c]["outl"]).astype(np.float32)
        oc = np.concatenate(
            [op.transpose(0, 2, 1, 3, 4).reshape(NBLK - 1, P, NST, D), ol[None]],
            axis=0,
        )
        # [31, p, st, d] -> rows s = st*128 + p
        oc = oc.transpose(0, 2, 1, 3).reshape(NBLK, SBK, D)
        bs = _blocks_for_core(c)
        barr = np.array([b for b, _ in bs])
        sarr = np.array([s for _, s in bs])
        o4[barr, sarr] = oc
    return out, res


def kernel(**inputs) -> np.ndarray:
    out, _ = _run(trace=False, **inputs)
    return out


def kernel_traced(mode=None, **inputs):
    """Returns (out, BassKernelResults) with HW profile info."""
    return _run(trace=True, **inputs)
